# revision 5
# baseline (speedup 1.0000x reference)
"""Dcls1d (dilated conv1d with learnable spacings) on 8 Trainium2 NeuronCores.

Problem: x (8, 256, 2048) f32; weight (256, 256, 16); P (1, 256, 256, 16);
bias (256,). A dense conv kernel (O=256, I=256, DKS=33) is built from
weight/P by linear interpolation at positions P, then conv1d(x, kern,
pad=16) + bias -> out (8, 256, 2048).

Strategy (data-parallel over batch, one batch element per core):
 - Host-fold (weight, P) into per-tap matmul weights. Active taps 13..19;
   taps 15/16/17 carry ~97.5% of the kernel energy, 14/18 ~1.15% each,
   13/19 are ~28-row sparse.
 - Mixed precision split by tap energy: the heavy taps (15/16/17) run
   fp16 (K=128 matmuls, 1 col/cycle); the light dense taps (14/18) run
   fp8-e4m3 in DoubleRow mode - one K=256 matmul per tap at the same
   216ns stream time as a single fp16 K=128 matmul, i.e. 2x MACs/cycle.
   With w*16 / x/16 scaling the fp8 products accumulate into the same
   fp32 PSUM bank as the fp16 taps. Measured end-to-end rel err ~6e-3
   (gate 2e-2); fp8 everywhere fails (3.6e-2) since the PE upcasts fp8
   to e6m3 (3 mantissa bits).
 - Sparse taps 13/19 pack their (tap, row) pairs - with host-pre-shifted
   x copies - into one K=128 fp16 strip matmul; the output bias rides
   along as one extra strip row (bias against a constant-1.0 x row), so
   no separate bias pass exists.
 - Per output tile (128 oc x 512 cols) a single PSUM bank accumulates
   9 matmuls (6 fp16 + 2 fp8-DR + 1 strip); the close is one PSUM->SBUF
   copy (alternating vector/scalar engines) and a store spread across
   the sync/scalar/gpsimd DMA rings.
 - PE warmup matmuls (vector-memset warm tile, not gpsimd: gpsimd's
   first instruction lands ~6us late) start the clock ramp during the
   fixed ~6.8us sequencer init + first DMA flight, so real matmuls
   start as soon as the lead x/kt chunks land.
"""

import numpy as np

try:
    import concourse  # noqa: F401
except ImportError:  # pragma: no cover - container fallback
    import sys

    sys.path.insert(0, "/opt/trn_rl_repo")

import concourse.bacc as bacc
import concourse.mybir as mybir
import concourse.tile as tile
import concourse.bass_utils as bass_utils

DKS = 33
PAD = 16
N, IC, LEN = 8, 256, 2048
OC = 256
KC = 16
N_CORES = 8
SLAB_W = 64   # taps with <= this many nonzero rows go to the packed strip
FP8_FRAC = 0.04  # per-tap energy fraction below which a dense tap runs fp8
N_WARM = 4
W_SCALE = np.float32(16.0)

TRACE = False  # test harness sets kernel_mod.TRACE = True to profile
LAST_EXEC_NS = None
LAST_TRACE_PATH = None

F16 = np.float16

_BUILD_CACHE = {}


def _to_e4m3(a):
    import ml_dtypes

    return np.clip(a, -240.0, 240.0).astype(ml_dtypes.float8_e4m3fn)


def _host_fold_kernel(weight, P):
    """Reproduce reference construct_kernel for the active taps only.

    Returns (dmin, ktaps) with ktaps[t, i, o] the lhsT-layout weights for
    tap d = dmin + t, in fp32 mirroring the reference arithmetic.
    """
    w = np.asarray(weight, dtype=np.float32)
    Pf32 = np.asarray(P, dtype=np.float32)
    Pp = Pf32 + np.float32(DKS // 2)
    Pf = np.floor(Pp)
    frac = (Pp - Pf)[0, 0]  # (IC, KC) - out-channel 0's fractional part
    P1 = Pf[0]  # (OC, IC, KC)

    dmin = max(0, int(P1.min()))
    dmax = min(DKS - 1, int(P1.max()) + 1)
    dd = np.arange(dmin, dmax + 1, dtype=np.float32)
    W1 = dd[:, None, None, None] == P1[None]
    W2 = dd[:, None, None, None] == (P1 + 1)[None]
    K = W1.astype(np.float32) + frac[None, None] * (
        W2.astype(np.float32) - W1.astype(np.float32)
    )
    kern = (w[None] * K).sum(-1)  # (T, OC, IC)
    ktaps = np.ascontiguousarray(kern.transpose(0, 2, 1))  # (T, IC, OC)
    return dmin, ktaps


def _classify_taps(ktaps):
    """Split taps: strip (sparse), fp8 (low-energy dense), fp16 (the rest)."""
    T = ktaps.shape[0]
    nzrows = [np.nonzero(np.any(ktaps[t] != 0, axis=1))[0] for t in range(T)]
    en = np.array([(ktaps[t] ** 2).sum() for t in range(T)])
    frac = en / max(en.sum(), 1e-30)
    strips = [(t, nzrows[t]) for t in range(T)
              if 0 < len(nzrows[t]) <= SLAB_W]
    strip_set = {t for t, _ in strips}
    rest = [t for t in range(T) if t not in strip_set and len(nzrows[t])]
    # fp8 set: lowest-energy taps while the cumulative energy fraction
    # stays small enough for the e4m3 error budget (~5% * sqrt(frac))
    fp8 = []
    cum = 0.0
    for t in sorted(rest, key=lambda t: frac[t]):
        if frac[t] <= FP8_FRAC and cum + frac[t] <= 2 * FP8_FRAC:
            fp8.append(t)
            cum += frac[t]
    fp8 = sorted(fp8)
    dense16 = sorted(t for t in rest if t not in fp8)
    return dense16, fp8, strips


def _build(T, nd16, nd8, strip_sizes):
    f32 = mybir.dt.float32
    f16 = mybir.dt.float16
    f8 = mybir.dt.float8e4
    DR = mybir.MatmulPerfMode.DoubleRow

    W = LEN + T - 1  # host-padded x width; tap t reads cols [t+c0, +512)
    n_tc = LEN // 512
    n_sg = len(strip_sizes)

    nc = bacc.Bacc("TRN2", target_bir_lowering=False, debug=False,
                   num_devices=N_CORES)
    x_d = nc.dram_tensor("x", (128, 2, W), f16, kind="ExternalInput")
    kt_d = nc.dram_tensor("kt", (128, 2, 2, nd16, 128), f16,
                          kind="ExternalInput")
    x8_d = nc.dram_tensor("x8", (128, 2, W), f8, kind="ExternalInput")
    kt8_d = nc.dram_tensor("kt8", (128, 2, 2, nd8, 128), f8,
                           kind="ExternalInput")
    xg_d = [nc.dram_tensor(f"xg{g}", (sp, LEN), f16, kind="ExternalInput")
            for g, sp in enumerate(strip_sizes)]
    kp_d = [nc.dram_tensor(f"kp{g}", (128, OC), f16, kind="ExternalInput")
            for g in range(n_sg)]
    y_d = nc.dram_tensor("out", (2, 128, LEN), f16, kind="ExternalOutput")

    SP_REAL = list(strip_sizes)

    with tile.TileContext(nc) as tc:
        with (
            tc.tile_pool(name="const", bufs=1) as cpool,
            tc.tile_pool(name="ps", bufs=8, space="PSUM") as pspool,
            tc.tile_pool(name="outp", bufs=4) as opool,
        ):
            xp = cpool.tile([128, 2, W], f16, tag="xp", name="xp")
            kt_t = cpool.tile([128, 2, 2, nd16, 128], f16, tag="kt",
                              name="kt")
            # fp8 operands: x8 mirrors xp's (k, ic-tile, col) layout so a
            # [:, :, c:c+512] slice is exactly the DoubleRow moving AP
            # (K=128 partitions x 2 k-tiles x 512 cols = K256 contraction)
            x8_t = cpool.tile([128, 2, W], f8, tag="x8", name="x8")
            kt8_t = cpool.tile([128, 2, 2, nd8, 128], f8, tag="kt8",
                               name="kt8")
            # strip operands padded to the full 128 partitions: a K<128
            # matmul streams at half SBUF bandwidth (measured 312ns vs
            # 216ns), so zero-fill the tail rows and run K=128
            xg_t = [cpool.tile([128, LEN], f16, tag=f"xg{g}", name=f"xg{g}")
                    for g in range(n_sg)]
            kp_t = [cpool.tile([128, OC], f16, tag=f"kp{g}", name=f"kp{g}")
                    for g in range(n_sg)]

            # PE warmup: starts the clock ramp during the fixed sequencer
            # init; memset on the vector engine (gpsimd's first op lands
            # ~6us late and would serialize the in-order PE queue).
            warm = cpool.tile([128, 512], f16, tag="warm")
            nc.vector.memset(warm[:], 0.0)
            wps = pspool.tile([64, 512], f32, tag="ps", name="warm_ps")
            for _ in range(N_WARM):
                nc.tensor.matmul(wps[:], warm[:, 0:64], warm[:],
                                 start=True, stop=True)

            # Input DMA. Each dma_start is a ~700ns DIRECT2D on the
            # issuing sequencer; descriptors then spray across the 16 hw
            # queues. Short descriptors are overhead-bound (~77ns each,
            # ~254GB/s for 1KB runs), so transfer whole contiguous
            # per-partition runs (4KB+) wherever the pipeline allows;
            # only the lead x chunk is column-sliced so the first real
            # matmul can start early.
            nc.sync.dma_start(xp[:, 0, 0:520], x_d.ap()[:, 0, 0:520])
            nc.sync.dma_start(xp[:, 0, 520:W], x_d.ap()[:, 0, 520:W])
            nc.sync.dma_start(x8_t[:], x8_d.ap())

            nc.scalar.dma_start(kt_t[:, 0, 0], kt_d.ap()[:, 0, 0])
            nc.scalar.dma_start(kt_t[:, 0, 1], kt_d.ap()[:, 0, 1])
            nc.scalar.dma_start(kt_t[:, 1], kt_d.ap()[:, 1])
            nc.scalar.dma_start(kt8_t[:], kt8_d.ap())
            for g in range(n_sg):
                nc.scalar.dma_start(kp_t[g][:], kp_d[g].ap())

            nc.gpsimd.dma_start(xp[:, 1], x_d.ap()[:, 1])
            for g in range(n_sg):
                nc.gpsimd.dma_start(xg_t[g][:SP_REAL[g]],
                                    xg_d[g].ap()[:SP_REAL[g]])

            for g, sp in enumerate(strip_sizes):
                if sp < 128:
                    nc.vector.memset(xg_t[g][sp:128, :], 0.0)

            ps = {}
            for tcn in range(n_tc):
                for oc in range(2):
                    ps[tcn, oc] = pspool.tile([128, 512], f32, tag="ps",
                                              name=f"ps_{tcn}_{oc}")

            def dense16_pass(ic, oc, start):  # ic0 phase
                for tcn in range(n_tc):
                    c0 = tcn * 512
                    for di in range(nd16):
                        o = DOFF16[di] + c0
                        nc.tensor.matmul(
                            ps[tcn, oc][:], kt_t[:, ic, oc, di, :],
                            xp[:, ic, o:o + 512],
                            start=(start and di == 0), stop=False,
                        )

            def tile_close(tcn, oc):
                c0 = tcn * 512
                ocs = slice(oc * 128, (oc + 1) * 128)
                last = (tcn == n_tc - 1 and oc == 1)
                # light taps: one K=256 fp8 DoubleRow matmul each
                for di in range(nd8):
                    o = DOFF8[di] + c0
                    nc.tensor.matmul(
                        ps[tcn, oc][:], kt8_t[:, :, oc, di, :],
                        x8_t[:, :, o:o + 512],
                        start=False, stop=False,
                        perf_mode=mybir.MatmulPerfMode.DoubleRow,
                    )
                for g in range(n_sg):
                    nc.tensor.matmul(
                        ps[tcn, oc][:], kp_t[g][:, ocs],
                        xg_t[g][:, c0:c0 + 512],
                        start=False, stop=(g == n_sg - 1),
                    )
                ot = opool.tile([128, 512], f16, tag="ot",
                                name=f"ot_{tcn}_{oc}")
                if not last:
                    if tcn % 2 == 0:
                        nc.vector.tensor_copy(ot[:], ps[tcn, oc][:])
                    else:
                        nc.scalar.activation(
                            ot[:], ps[tcn, oc][:],
                            mybir.ActivationFunctionType.Copy)
                    deng = (nc.gpsimd, nc.sync, nc.scalar)[(oc * n_tc + tcn) % 3]
                    deng.dma_start(y_d.ap()[oc][:, c0:c0 + 512], ot[:])
                else:
                    # split the final copy+store to trim the tail
                    nc.vector.tensor_copy(ot[:, 0:256], ps[tcn, oc][:, 0:256])
                    nc.scalar.activation(
                        ot[:, 256:512], ps[tcn, oc][:, 256:512],
                        mybir.ActivationFunctionType.Copy)
                    nc.gpsimd.dma_start(
                        y_d.ap()[oc][:, c0:c0 + 256], ot[:, 0:256])
                    nc.sync.dma_start(
                        y_d.ap()[oc][:, c0 + 256:c0 + 512], ot[:, 256:512])

            # Phase 1: heavy-tap ic0 matmuls while ic1/fp8/strip inputs
            # stream in. Phase 2: per tile, heavy-tap ic1 + fp8 taps +
            # strip + close, so stores spread across the back half.
            dense16_pass(0, 0, True)
            dense16_pass(0, 1, True)
            for oc in range(2):
                for tcn in range(n_tc):
                    c0 = tcn * 512
                    for di in range(nd16):
                        o = DOFF16[di] + c0
                        nc.tensor.matmul(
                            ps[tcn, oc][:], kt_t[:, 1, oc, di, :],
                            xp[:, 1, o:o + 512],
                            start=False, stop=False,
                        )
                    tile_close(tcn, oc)

    nc.compile()
    return nc


def kernel(x, weight, P, bias):
    global LAST_EXEC_NS, LAST_TRACE_PATH, DOFF16, DOFF8
    x = np.ascontiguousarray(np.asarray(x, dtype=np.float32))
    bias = np.asarray(bias, dtype=np.float32)

    dmin, ktaps = _host_fold_kernel(weight, P)
    T = ktaps.shape[0]
    dense16, dense8, strips = _classify_taps(ktaps)
    nd16, nd8 = len(dense16), len(dense8)
    assert nd16 >= 1, "degenerate kernel"

    # strip groups: (tap, row) pairs + one bias row, <= 128 rows per group
    rows = [(t, int(r)) for t, rr in strips for r in rr] + [(-1, -1)]
    groups = [rows[i:i + 128] for i in range(0, len(rows), 128)]
    # pad each group to a 32-aligned row count: the on-device zero-fill
    # of the remaining partitions must start at a 32-aligned partition
    groups = [g + [(-2, -1)] * (-len(g) % 32) for g in groups]
    strip_sizes = tuple(len(g) for g in groups)

    DOFF16 = list(dense16)  # tap column offsets used at emission time
    DOFF8 = list(dense8)

    key = (T, tuple(dense16), tuple(dense8),
           tuple(t for t, _ in rows[:-1]), strip_sizes)
    if key not in _BUILD_CACHE:
        _BUILD_CACHE[key] = _build(T, nd16, nd8, strip_sizes)
    nc = _BUILD_CACHE[key]

    # host-side input packing -------------------------------------------
    W = LEN + T - 1
    zl = max(0, PAD - dmin)
    xs = max(0, dmin - PAD)
    xn = min(LEN - xs, W - zl)
    xpad = np.zeros((N_CORES, 2, 128, W), dtype=np.float32)
    xpad[:, :, :, zl:zl + xn] = (
        x.reshape(N_CORES, 2, 128, LEN)[:, :, :, xs:xs + xn])

    xT = np.ascontiguousarray(xpad.transpose(0, 2, 1, 3))  # (c, 128, 2, W)
    x16 = xT.astype(F16)
    x8 = _to_e4m3(xT / W_SCALE).view(np.uint8)
    kt = np.ascontiguousarray(
        ktaps[dense16].reshape(nd16, 2, 128, 2, 128).transpose(2, 1, 3, 0, 4)
    ).astype(F16)
    # kt8[k, ic-tile, oc, tap, m] = ktaps[tap][ic_tile*128+k, oc*128+m]*16
    kt8 = _to_e4m3(
        np.ascontiguousarray(
            ktaps[dense8].reshape(nd8, 2, 128, 2, 128).transpose(2, 1, 3, 0, 4)
        ) * W_SCALE
    ).view(np.uint8)

    flat_x = xpad.reshape(N_CORES, 256, W)
    kps, xgs = [], []
    for g in groups:
        sp = len(g)
        kp = np.zeros((128, OC), dtype=np.float32)
        xg = np.zeros((N_CORES, sp, LEN), dtype=np.float32)
        for p, (t_sp, r) in enumerate(g):
            if t_sp == -2:  # alignment padding, stays zero
                continue
            if t_sp < 0:  # bias row
                kp[p] = bias
                xg[:, p] = 1.0
            else:
                kp[p] = ktaps[t_sp][r]
                xg[:, p] = flat_x[:, r, t_sp:t_sp + LEN]
        kps.append(kp.astype(F16))
        xgs.append(xg.astype(F16))

    in_maps = []
    for c in range(N_CORES):
        m = {"x": x16[c], "x8": x8[c], "kt": kt, "kt8": kt8}
        for g in range(len(groups)):
            m[f"kp{g}"] = kps[g]
            m[f"xg{g}"] = xgs[g][c]
        in_maps.append(m)

    kwargs = {}
    bass_utils.upload_artifacts = lambda tmpdir: tmpdir
    if TRACE:
        kwargs["trace"] = True
    res = None
    for attempt in range(3):
        try:
            res = bass_utils.run_bass_kernel_spmd(
                nc, in_maps, core_ids=list(range(N_CORES)), **kwargs
            )
            break
        except Exception:
            # occasional transient NRT_EXEC_UNIT_UNRECOVERABLE on this
            # fabric; give the device a moment to recover, then retry
            if attempt == 2:
                raise
            import time
            time.sleep(3.0)
    if TRACE:
        LAST_EXEC_NS = res.exec_time_ns
        if res.instructions_and_trace is not None:
            LAST_TRACE_PATH = res.instructions_and_trace[1]

    out = np.empty((N, OC, LEN), dtype=np.float32)
    for c in range(N_CORES):
        out[c] = res.results[c]["out"].reshape(OC, LEN).astype(np.float32)
    return out


# revision 7
# speedup vs baseline: 1.0715x; 1.0715x over previous
"""Dcls1d (dilated conv1d with learnable spacings) on 8 Trainium2 NeuronCores.

Problem: x (8, 256, 2048) f32; weight (256, 256, 16); P (1, 256, 256, 16);
bias (256,). A dense conv kernel (O=256, I=256, DKS=33) is built from
weight/P by linear interpolation at positions P, then conv1d(x, kern,
pad=16) + bias -> out (8, 256, 2048).

Strategy (data-parallel over batch, one batch element per core):
 - Host-fold (weight, P) into per-tap matmul weights. Active taps 13..19;
   taps 15/16/17 carry ~97.5% of the kernel energy, 14/18 ~1.15% each,
   13/19 are ~28-row sparse.
 - Mixed precision split by tap energy: the heavy taps (15/16/17) run
   fp16 (K=128 matmuls, 1 col/cycle); the light dense taps (14/18) run
   fp8-e4m3 in DoubleRow mode - one K=256 matmul per tap at the same
   216ns stream time as a single fp16 K=128 matmul, i.e. 2x MACs/cycle.
   With w*16 / x/16 scaling the fp8 products accumulate into the same
   fp32 PSUM bank as the fp16 taps. Measured end-to-end rel err ~6e-3
   (gate 2e-2); fp8 everywhere fails (3.6e-2) since the PE upcasts fp8
   to e6m3 (3 mantissa bits).
 - Sparse taps 13/19 pack their (tap, row) pairs - with host-pre-shifted
   x copies - into one K=128 fp16 strip matmul; the output bias rides
   along as one extra strip row (bias against a constant-1.0 x row), so
   no separate bias pass exists.
 - Per output tile (128 oc x 512 cols) a single PSUM bank accumulates
   9 matmuls (6 fp16 + 2 fp8-DR + 1 strip); the close is one PSUM->SBUF
   copy (alternating vector/scalar engines) and a store spread across
   the sync/scalar/gpsimd DMA rings.
 - PE warmup matmuls (vector-memset warm tile, not gpsimd: gpsimd's
   first instruction lands ~6us late) start the clock ramp during the
   fixed ~6.8us sequencer init + first DMA flight, so real matmuls
   start as soon as the lead x/kt chunks land.
"""

import numpy as np

try:
    import concourse  # noqa: F401
except ImportError:  # pragma: no cover - container fallback
    import sys

    sys.path.insert(0, "/opt/trn_rl_repo")

import concourse.bacc as bacc
import concourse.mybir as mybir
import concourse.tile as tile
import concourse.bass_utils as bass_utils

DKS = 33
PAD = 16
N, IC, LEN = 8, 256, 2048
OC = 256
KC = 16
N_CORES = 8
SLAB_W = 64   # taps with <= this many nonzero rows go to the packed strip
FP8_FRAC = 0.04  # per-tap energy fraction below which a dense tap runs fp8
N_WARM = 3
W_SCALE = np.float32(16.0)

TRACE = False  # test harness sets kernel_mod.TRACE = True to profile
LAST_EXEC_NS = None
LAST_TRACE_PATH = None

F16 = np.float16

_BUILD_CACHE = {}


def _to_e4m3(a):
    import ml_dtypes

    return np.clip(a, -240.0, 240.0).astype(ml_dtypes.float8_e4m3fn)


def _host_fold_kernel(weight, P):
    """Reproduce reference construct_kernel for the active taps only.

    Returns (dmin, ktaps) with ktaps[t, i, o] the lhsT-layout weights for
    tap d = dmin + t, in fp32 mirroring the reference arithmetic.
    """
    w = np.asarray(weight, dtype=np.float32)
    Pf32 = np.asarray(P, dtype=np.float32)
    Pp = Pf32 + np.float32(DKS // 2)
    Pf = np.floor(Pp)
    frac = (Pp - Pf)[0, 0]  # (IC, KC) - out-channel 0's fractional part
    P1 = Pf[0]  # (OC, IC, KC)

    dmin = max(0, int(P1.min()))
    dmax = min(DKS - 1, int(P1.max()) + 1)
    dd = np.arange(dmin, dmax + 1, dtype=np.float32)
    W1 = dd[:, None, None, None] == P1[None]
    W2 = dd[:, None, None, None] == (P1 + 1)[None]
    K = W1.astype(np.float32) + frac[None, None] * (
        W2.astype(np.float32) - W1.astype(np.float32)
    )
    kern = (w[None] * K).sum(-1)  # (T, OC, IC)
    ktaps = np.ascontiguousarray(kern.transpose(0, 2, 1))  # (T, IC, OC)
    return dmin, ktaps


def _classify_taps(ktaps):
    """Split taps: strip (sparse), fp8 (low-energy dense), fp16 (the rest)."""
    T = ktaps.shape[0]
    nzrows = [np.nonzero(np.any(ktaps[t] != 0, axis=1))[0] for t in range(T)]
    en = np.array([(ktaps[t] ** 2).sum() for t in range(T)])
    frac = en / max(en.sum(), 1e-30)
    strips = [(t, nzrows[t]) for t in range(T)
              if 0 < len(nzrows[t]) <= SLAB_W]
    strip_set = {t for t, _ in strips}
    rest = [t for t in range(T) if t not in strip_set and len(nzrows[t])]
    # fp8 set: lowest-energy taps while the cumulative energy fraction
    # stays small enough for the e4m3 error budget (~5% * sqrt(frac))
    fp8 = []
    cum = 0.0
    for t in sorted(rest, key=lambda t: frac[t]):
        if frac[t] <= FP8_FRAC and cum + frac[t] <= 2 * FP8_FRAC:
            fp8.append(t)
            cum += frac[t]
    fp8 = sorted(fp8)
    dense16 = sorted(t for t in rest if t not in fp8)
    return dense16, fp8, strips


def _build(T, nd16, nd8, strip_sizes):
    f32 = mybir.dt.float32
    f16 = mybir.dt.float16
    f8 = mybir.dt.float8e4
    DR = mybir.MatmulPerfMode.DoubleRow

    W = LEN + T - 1  # host-padded x width; tap t reads cols [t+c0, +512)
    n_tc = LEN // 512
    n_sg = len(strip_sizes)

    nc = bacc.Bacc("TRN2", target_bir_lowering=False, debug=False,
                   num_devices=N_CORES)
    x_d = nc.dram_tensor("x", (128, 2, W), f16, kind="ExternalInput")
    kt_d = nc.dram_tensor("kt", (128, 2, 2, nd16, 128), f16,
                          kind="ExternalInput")
    x8_d = nc.dram_tensor("x8", (128, 2, W), f8, kind="ExternalInput")
    kt8_d = nc.dram_tensor("kt8", (128, 2, 2, nd8, 128), f8,
                           kind="ExternalInput")
    xg_d = [nc.dram_tensor(f"xg{g}", (sp, LEN), f16, kind="ExternalInput")
            for g, sp in enumerate(strip_sizes)]
    kp_d = [nc.dram_tensor(f"kp{g}", (128, OC), f16, kind="ExternalInput")
            for g in range(n_sg)]
    y_d = nc.dram_tensor("out", (2, 128, LEN), f16, kind="ExternalOutput")

    SP_REAL = list(strip_sizes)

    with tile.TileContext(nc) as tc:
        with (
            tc.tile_pool(name="const", bufs=1) as cpool,
            tc.tile_pool(name="ps", bufs=8, space="PSUM") as pspool,
            tc.tile_pool(name="outp", bufs=4) as opool,
        ):
            xp = cpool.tile([128, 2, W], f16, tag="xp", name="xp")
            kt_t = cpool.tile([128, 2, 2, nd16, 128], f16, tag="kt",
                              name="kt")
            # fp8 operands: x8 mirrors xp's (k, ic-tile, col) layout so a
            # [:, :, c:c+512] slice is exactly the DoubleRow moving AP
            # (K=128 partitions x 2 k-tiles x 512 cols = K256 contraction)
            x8_t = cpool.tile([128, 2, W], f8, tag="x8", name="x8")
            kt8_t = cpool.tile([128, 2, 2, nd8, 128], f8, tag="kt8",
                               name="kt8")
            # strip operands padded to the full 128 partitions: a K<128
            # matmul streams at half SBUF bandwidth (measured 312ns vs
            # 216ns), so zero-fill the tail rows and run K=128
            xg_t = [cpool.tile([128, LEN], f16, tag=f"xg{g}", name=f"xg{g}")
                    for g in range(n_sg)]
            kp_t = [cpool.tile([128, OC], f16, tag=f"kp{g}", name=f"kp{g}")
                    for g in range(n_sg)]

            # PE warmup: starts the clock ramp during the fixed sequencer
            # init; memset on the vector engine (gpsimd's first op lands
            # ~6us late and would serialize the in-order PE queue).
            warm = cpool.tile([128, 512], f16, tag="warm")
            nc.vector.memset(warm[:], 0.0)
            wps = pspool.tile([64, 512], f32, tag="ps", name="warm_ps")
            for _ in range(N_WARM):
                nc.tensor.matmul(wps[:], warm[:, 0:64], warm[:],
                                 start=True, stop=True)

            # Input DMA. Each dma_start is a ~700ns DIRECT2D on the
            # issuing sequencer; descriptors then spray across the 16 hw
            # queues. Short descriptors are overhead-bound (~77ns each,
            # ~254GB/s for 1KB runs), so transfer whole contiguous
            # per-partition runs (4KB+) wherever the pipeline allows;
            # only the lead x chunk is column-sliced so the first real
            # matmul can start early.
            # The 16 hw queues process descriptors in doorbell order, so
            # issue transfers in strict need-order: all x pieces in
            # column order on the sync ring, all weights/strip operands
            # in phase order on the scalar ring. gpsimd's ring (slow
            # ~650ns DIRECT2Ds) is left for phase-2 stores only.
            nc.sync.dma_start(xp[:, 0, 0:1040], x_d.ap()[:, 0, 0:1040])
            nc.sync.dma_start(xp[:, 0, 1040:W], x_d.ap()[:, 0, 1040:W])
            nc.sync.dma_start(xp[:, 1], x_d.ap()[:, 1])
            nc.sync.dma_start(x8_t[:], x8_d.ap())

            nc.scalar.dma_start(kt_t[:, 0, 0], kt_d.ap()[:, 0, 0])
            nc.scalar.dma_start(kt_t[:, 0, 1], kt_d.ap()[:, 0, 1])
            nc.scalar.dma_start(kt_t[:, 1], kt_d.ap()[:, 1])
            nc.scalar.dma_start(kt8_t[:], kt8_d.ap())
            for g in range(n_sg):
                nc.scalar.dma_start(kp_t[g][:], kp_d[g].ap())
                nc.scalar.dma_start(xg_t[g][:SP_REAL[g]],
                                    xg_d[g].ap()[:SP_REAL[g]])

            for g, sp in enumerate(strip_sizes):
                if sp < 128:
                    nc.vector.memset(xg_t[g][sp:128, :], 0.0)

            ps = {}
            for tcn in range(n_tc):
                for oc in range(2):
                    ps[tcn, oc] = pspool.tile([128, 512], f32, tag="ps",
                                              name=f"ps_{tcn}_{oc}")

            def dense16_pass(ic, oc, start):  # ic0 phase
                for tcn in range(n_tc):
                    c0 = tcn * 512
                    for di in range(nd16):
                        o = DOFF16[di] + c0
                        nc.tensor.matmul(
                            ps[tcn, oc][:], kt_t[:, ic, oc, di, :],
                            xp[:, ic, o:o + 512],
                            start=(start and di == 0), stop=False,
                        )

            def tile_close(tcn, oc):
                c0 = tcn * 512
                ocs = slice(oc * 128, (oc + 1) * 128)
                last = (tcn == n_tc - 1 and oc == 1)
                # light taps: one K=256 fp8 DoubleRow matmul each
                for di in range(nd8):
                    o = DOFF8[di] + c0
                    nc.tensor.matmul(
                        ps[tcn, oc][:], kt8_t[:, :, oc, di, :],
                        x8_t[:, :, o:o + 512],
                        start=False, stop=False,
                        perf_mode=mybir.MatmulPerfMode.DoubleRow,
                    )
                for g in range(n_sg):
                    nc.tensor.matmul(
                        ps[tcn, oc][:], kp_t[g][:, ocs],
                        xg_t[g][:, c0:c0 + 512],
                        start=False, stop=(g == n_sg - 1),
                    )
                ot = opool.tile([128, 512], f16, tag="ot",
                                name=f"ot_{tcn}_{oc}")
                if not last:
                    if tcn % 2 == 0:
                        nc.vector.tensor_copy(ot[:], ps[tcn, oc][:])
                    else:
                        nc.scalar.activation(
                            ot[:], ps[tcn, oc][:],
                            mybir.ActivationFunctionType.Copy)
                    deng = (nc.gpsimd, nc.sync, nc.scalar)[(oc * n_tc + tcn) % 3]
                    deng.dma_start(y_d.ap()[oc][:, c0:c0 + 512], ot[:])
                else:
                    # split the final copy+store to trim the tail
                    nc.vector.tensor_copy(ot[:, 0:256], ps[tcn, oc][:, 0:256])
                    nc.scalar.activation(
                        ot[:, 256:512], ps[tcn, oc][:, 256:512],
                        mybir.ActivationFunctionType.Copy)
                    nc.gpsimd.dma_start(
                        y_d.ap()[oc][:, c0:c0 + 256], ot[:, 0:256])
                    nc.sync.dma_start(
                        y_d.ap()[oc][:, c0 + 256:c0 + 512], ot[:, 256:512])

            # Phase 1: heavy-tap ic0 matmuls while ic1/fp8/strip inputs
            # stream in. Phase 2: per tile, heavy-tap ic1 + fp8 taps +
            # strip + close, so stores spread across the back half.
            dense16_pass(0, 0, True)
            dense16_pass(0, 1, True)
            for oc in range(2):
                for tcn in range(n_tc):
                    c0 = tcn * 512
                    for di in range(nd16):
                        o = DOFF16[di] + c0
                        nc.tensor.matmul(
                            ps[tcn, oc][:], kt_t[:, 1, oc, di, :],
                            xp[:, 1, o:o + 512],
                            start=False, stop=False,
                        )
                    tile_close(tcn, oc)

    nc.compile()
    return nc


def kernel(x, weight, P, bias):
    global LAST_EXEC_NS, LAST_TRACE_PATH, DOFF16, DOFF8
    x = np.ascontiguousarray(np.asarray(x, dtype=np.float32))
    bias = np.asarray(bias, dtype=np.float32)

    dmin, ktaps = _host_fold_kernel(weight, P)
    T = ktaps.shape[0]
    dense16, dense8, strips = _classify_taps(ktaps)
    nd16, nd8 = len(dense16), len(dense8)
    assert nd16 >= 1, "degenerate kernel"

    # strip groups: (tap, row) pairs + one bias row, <= 128 rows per group
    rows = [(t, int(r)) for t, rr in strips for r in rr] + [(-1, -1)]
    groups = [rows[i:i + 128] for i in range(0, len(rows), 128)]
    # pad each group to a 32-aligned row count: the on-device zero-fill
    # of the remaining partitions must start at a 32-aligned partition
    groups = [g + [(-2, -1)] * (-len(g) % 32) for g in groups]
    strip_sizes = tuple(len(g) for g in groups)

    DOFF16 = list(dense16)  # tap column offsets used at emission time
    DOFF8 = list(dense8)

    key = (T, tuple(dense16), tuple(dense8),
           tuple(t for t, _ in rows[:-1]), strip_sizes)
    if key not in _BUILD_CACHE:
        _BUILD_CACHE[key] = _build(T, nd16, nd8, strip_sizes)
    nc = _BUILD_CACHE[key]

    # host-side input packing -------------------------------------------
    W = LEN + T - 1
    zl = max(0, PAD - dmin)
    xs = max(0, dmin - PAD)
    xn = min(LEN - xs, W - zl)
    xpad = np.zeros((N_CORES, 2, 128, W), dtype=np.float32)
    xpad[:, :, :, zl:zl + xn] = (
        x.reshape(N_CORES, 2, 128, LEN)[:, :, :, xs:xs + xn])

    xT = np.ascontiguousarray(xpad.transpose(0, 2, 1, 3))  # (c, 128, 2, W)
    x16 = xT.astype(F16)
    x8 = _to_e4m3(xT / W_SCALE).view(np.uint8)
    kt = np.ascontiguousarray(
        ktaps[dense16].reshape(nd16, 2, 128, 2, 128).transpose(2, 1, 3, 0, 4)
    ).astype(F16)
    # kt8[k, ic-tile, oc, tap, m] = ktaps[tap][ic_tile*128+k, oc*128+m]*16
    kt8 = _to_e4m3(
        np.ascontiguousarray(
            ktaps[dense8].reshape(nd8, 2, 128, 2, 128).transpose(2, 1, 3, 0, 4)
        ) * W_SCALE
    ).view(np.uint8)

    flat_x = xpad.reshape(N_CORES, 256, W)
    kps, xgs = [], []
    for g in groups:
        sp = len(g)
        kp = np.zeros((128, OC), dtype=np.float32)
        xg = np.zeros((N_CORES, sp, LEN), dtype=np.float32)
        for p, (t_sp, r) in enumerate(g):
            if t_sp == -2:  # alignment padding, stays zero
                continue
            if t_sp < 0:  # bias row
                kp[p] = bias
                xg[:, p] = 1.0
            else:
                kp[p] = ktaps[t_sp][r]
                xg[:, p] = flat_x[:, r, t_sp:t_sp + LEN]
        kps.append(kp.astype(F16))
        xgs.append(xg.astype(F16))

    in_maps = []
    for c in range(N_CORES):
        m = {"x": x16[c], "x8": x8[c], "kt": kt, "kt8": kt8}
        for g in range(len(groups)):
            m[f"kp{g}"] = kps[g]
            m[f"xg{g}"] = xgs[g][c]
        in_maps.append(m)

    kwargs = {}
    bass_utils.upload_artifacts = lambda tmpdir: tmpdir
    if TRACE:
        kwargs["trace"] = True
    res = None
    for attempt in range(3):
        try:
            res = bass_utils.run_bass_kernel_spmd(
                nc, in_maps, core_ids=list(range(N_CORES)), **kwargs
            )
            break
        except Exception:
            # occasional transient NRT_EXEC_UNIT_UNRECOVERABLE on this
            # fabric; give the device a moment to recover, then retry
            if attempt == 2:
                raise
            import time
            time.sleep(3.0)
    if TRACE:
        LAST_EXEC_NS = res.exec_time_ns
        if res.instructions_and_trace is not None:
            LAST_TRACE_PATH = res.instructions_and_trace[1]

    out = np.empty((N, OC, LEN), dtype=np.float32)
    for c in range(N_CORES):
        out[c] = res.results[c]["out"].reshape(OC, LEN).astype(np.float32)
    return out


# revision 12
# speedup vs baseline: 1.0793x; 1.0073x over previous
"""Dcls1d (dilated conv1d with learnable spacings) on 8 Trainium2 NeuronCores.

Problem: x (8, 256, 2048) f32; weight (256, 256, 16); P (1, 256, 256, 16);
bias (256,). A dense conv kernel (O=256, I=256, DKS=33) is built from
weight/P by linear interpolation at positions P, then conv1d(x, kern,
pad=16) + bias -> out (8, 256, 2048).

Strategy (data-parallel over batch, one batch element per core):
 - Host-fold (weight, P) into per-tap matmul weights. Active taps 13..19;
   taps 15/16/17 carry ~97.5% of the kernel energy, 14/18 ~1.15% each,
   13/19 are ~28-row sparse.
 - Mixed precision split by tap energy: the heavy taps (15/16/17) run
   fp16 (K=128 matmuls, 1 col/cycle); the light dense taps (14/18) run
   fp8-e4m3 in DoubleRow mode - one K=256 matmul per tap at the same
   216ns stream time as a single fp16 K=128 matmul, i.e. 2x MACs/cycle.
   With w*16 / x/16 scaling the fp8 products accumulate into the same
   fp32 PSUM bank as the fp16 taps. Measured end-to-end rel err ~6e-3
   (gate 2e-2); fp8 everywhere fails (3.6e-2) since the PE upcasts fp8
   to e6m3 (3 mantissa bits).
 - Sparse taps 13/19 pack their (tap, row) pairs - with host-pre-shifted
   x copies - into one K=128 fp16 strip matmul; the output bias rides
   along as one extra strip row (bias against a constant-1.0 x row), so
   no separate bias pass exists.
 - Per output tile (128 oc x 512 cols) a single PSUM bank accumulates
   9 matmuls (6 fp16 + 2 fp8-DR + 1 strip); the close is one PSUM->SBUF
   copy (alternating vector/scalar engines) and a store spread across
   the sync/scalar/gpsimd DMA rings.
 - PE warmup matmuls (vector-memset warm tile, not gpsimd: gpsimd's
   first instruction lands ~6us late) start the clock ramp during the
   fixed ~6.8us sequencer init + first DMA flight, so real matmuls
   start as soon as the lead x/kt chunks land.
"""

import numpy as np

try:
    import concourse  # noqa: F401
except ImportError:  # pragma: no cover - container fallback
    import sys

    sys.path.insert(0, "/opt/trn_rl_repo")

import concourse.bacc as bacc
import concourse.mybir as mybir
import concourse.tile as tile
import concourse.bass_utils as bass_utils

DKS = 33
PAD = 16
N, IC, LEN = 8, 256, 2048
OC = 256
KC = 16
N_CORES = 8
SLAB_W = 64   # taps with <= this many nonzero rows go to the packed strip
FP8_FRAC = 0.04  # per-tap energy fraction below which a dense tap runs fp8
N_WARM = 2
W_SCALE = np.float32(16.0)

TRACE = False  # test harness sets kernel_mod.TRACE = True to profile
LAST_EXEC_NS = None
LAST_TRACE_PATH = None

F16 = np.float16

_BUILD_CACHE = {}


def _to_e4m3(a):
    import ml_dtypes

    return np.clip(a, -240.0, 240.0).astype(ml_dtypes.float8_e4m3fn)


def _host_fold_kernel(weight, P):
    """Reproduce reference construct_kernel for the active taps only.

    Returns (dmin, ktaps) with ktaps[t, i, o] the lhsT-layout weights for
    tap d = dmin + t, in fp32 mirroring the reference arithmetic.
    """
    w = np.asarray(weight, dtype=np.float32)
    Pf32 = np.asarray(P, dtype=np.float32)
    Pp = Pf32 + np.float32(DKS // 2)
    Pf = np.floor(Pp)
    frac = (Pp - Pf)[0, 0]  # (IC, KC) - out-channel 0's fractional part
    P1 = Pf[0]  # (OC, IC, KC)

    dmin = max(0, int(P1.min()))
    dmax = min(DKS - 1, int(P1.max()) + 1)
    dd = np.arange(dmin, dmax + 1, dtype=np.float32)
    W1 = dd[:, None, None, None] == P1[None]
    W2 = dd[:, None, None, None] == (P1 + 1)[None]
    K = W1.astype(np.float32) + frac[None, None] * (
        W2.astype(np.float32) - W1.astype(np.float32)
    )
    kern = (w[None] * K).sum(-1)  # (T, OC, IC)
    ktaps = np.ascontiguousarray(kern.transpose(0, 2, 1))  # (T, IC, OC)
    return dmin, ktaps


def _classify_taps(ktaps):
    """Split taps: strip (sparse), fp8 (low-energy dense), fp16 (the rest)."""
    T = ktaps.shape[0]
    nzrows = [np.nonzero(np.any(ktaps[t] != 0, axis=1))[0] for t in range(T)]
    en = np.array([(ktaps[t] ** 2).sum() for t in range(T)])
    frac = en / max(en.sum(), 1e-30)
    strips = [(t, nzrows[t]) for t in range(T)
              if 0 < len(nzrows[t]) <= SLAB_W]
    strip_set = {t for t, _ in strips}
    rest = [t for t in range(T) if t not in strip_set and len(nzrows[t])]
    # fp8 set: lowest-energy taps while the cumulative energy fraction
    # stays small enough for the e4m3 error budget (~5% * sqrt(frac))
    fp8 = []
    cum = 0.0
    for t in sorted(rest, key=lambda t: frac[t]):
        if frac[t] <= FP8_FRAC and cum + frac[t] <= 2 * FP8_FRAC:
            fp8.append(t)
            cum += frac[t]
    fp8 = sorted(fp8)
    dense16 = sorted(t for t in rest if t not in fp8)
    return dense16, fp8, strips


def _build(T, nd16, nd8, strip_sizes):
    f32 = mybir.dt.float32
    f16 = mybir.dt.float16
    f8 = mybir.dt.float8e4
    DR = mybir.MatmulPerfMode.DoubleRow

    W = LEN + T - 1  # host-padded x width; tap t reads cols [t+c0, +512)
    n_tc = LEN // 512
    n_sg = len(strip_sizes)

    nc = bacc.Bacc("TRN2", target_bir_lowering=False, debug=False,
                   num_devices=N_CORES)
    x_d = nc.dram_tensor("x", (128, 2, W), f16, kind="ExternalInput")
    kt_d = nc.dram_tensor("kt", (128, 2, 2, nd16, 128), f16,
                          kind="ExternalInput")
    kt8_d = nc.dram_tensor("kt8", (128, 2, 2, nd8, 128), f8,
                           kind="ExternalInput")
    xg_d = [nc.dram_tensor(f"xg{g}", (sp, LEN), f16, kind="ExternalInput")
            for g, sp in enumerate(strip_sizes)]
    kp_d = [nc.dram_tensor(f"kp{g}", (128, OC), f16, kind="ExternalInput")
            for g in range(n_sg)]
    y_d = nc.dram_tensor("out", (2, 128, LEN), f16, kind="ExternalOutput")

    SP_REAL = list(strip_sizes)

    with tile.TileContext(nc) as tc:
        with (
            tc.tile_pool(name="const", bufs=1) as cpool,
            tc.tile_pool(name="ps", bufs=8, space="PSUM") as pspool,
            tc.tile_pool(name="outp", bufs=4) as opool,
        ):
            xp = cpool.tile([128, 2, W], f16, tag="xp", name="xp")
            kt_t = cpool.tile([128, 2, 2, nd16, 128], f16, tag="kt",
                              name="kt")
            # fp8 operands: x8 mirrors xp's (k, ic-tile, col) layout so a
            # [:, :, c:c+512] slice is exactly the DoubleRow moving AP
            # (K=128 partitions x 2 k-tiles x 512 cols = K256 contraction)
            x8_t = cpool.tile([128, 2, W], f8, tag="x8", name="x8")
            kt8_t = cpool.tile([128, 2, 2, nd8, 128], f8, tag="kt8",
                               name="kt8")
            # strip operands padded to the full 128 partitions: a K<128
            # matmul streams at half SBUF bandwidth (measured 312ns vs
            # 216ns), so zero-fill the tail rows and run K=128
            xg_t = [cpool.tile([128, LEN], f16, tag=f"xg{g}", name=f"xg{g}")
                    for g in range(n_sg)]
            kp_t = [cpool.tile([128, OC], f16, tag=f"kp{g}", name=f"kp{g}")
                    for g in range(n_sg)]

            # PE warmup: starts the clock ramp during the fixed sequencer
            # init; memset on the vector engine (gpsimd's first op lands
            # ~6us late and would serialize the in-order PE queue).
            warm = cpool.tile([128, 512], f16, tag="warm")
            nc.vector.memset(warm[:], 0.0)
            wps = pspool.tile([64, 512], f32, tag="ps", name="warm_ps")
            for _ in range(N_WARM):
                nc.tensor.matmul(wps[:], warm[:, 0:64], warm[:],
                                 start=True, stop=True)

            # Input DMA. Each dma_start is a ~700ns DIRECT2D on the
            # issuing sequencer; descriptors then spray across the 16 hw
            # queues. Short descriptors are overhead-bound (~77ns each,
            # ~254GB/s for 1KB runs), so transfer whole contiguous
            # per-partition runs (4KB+) wherever the pipeline allows;
            # only the lead x chunk is column-sliced so the first real
            # matmul can start early.
            # Only 2 hw DGE rings exist (sync=SP, scalar=Activation) plus
            # gpsimd's software DGE; a ring serializes its own transfers
            # (~130GB/s with a small in-flight window), so balance the
            # three rings and keep each in strict need-order.
            nc.sync.dma_start(xp[:, 0, 0:520], x_d.ap()[:, 0, 0:520])
            nc.sync.dma_start(xp[:, 0, 520:1040], x_d.ap()[:, 0, 520:1040])
            nc.sync.dma_start(xp[:, 1, 1027:W], x_d.ap()[:, 1, 1027:W])

            nc.scalar.dma_start(kt_t[:, 0, 0], kt_d.ap()[:, 0, 0])
            nc.scalar.dma_start(kt_t[:, 0, 1], kt_d.ap()[:, 0, 1])
            nc.scalar.dma_start(kt_t[:, 1], kt_d.ap()[:, 1])
            nc.scalar.dma_start(kt8_t[:], kt8_d.ap())
            for g in range(n_sg):
                nc.scalar.dma_start(kp_t[g][:], kp_d[g].ap())

            nc.gpsimd.dma_start(xp[:, 0, 1040:1560], x_d.ap()[:, 0, 1040:1560])
            nc.gpsimd.dma_start(xp[:, 0, 1560:W], x_d.ap()[:, 0, 1560:W])
            nc.gpsimd.dma_start(xp[:, 1, 0:1027], x_d.ap()[:, 1, 0:1027])
            for g in range(n_sg):
                nc.gpsimd.dma_start(xg_t[g][:SP_REAL[g]],
                                    xg_d[g].ap()[:SP_REAL[g]])

            # x8 = x/16 in e4m3, produced on-chip on the idle vector and
            # scalar engines instead of shipping 0.5MB more over DMA
            nc.vector.tensor_scalar_mul(x8_t[:, 0], xp[:, 0],
                                        1.0 / float(W_SCALE))
            nc.scalar.activation(x8_t[:, 1], xp[:, 1],
                                 mybir.ActivationFunctionType.Copy,
                                 scale=1.0 / float(W_SCALE))

            for g, sp in enumerate(strip_sizes):
                if sp < 128:
                    nc.vector.memset(xg_t[g][sp:128, :], 0.0)

            ps = {}
            for tcn in range(n_tc):
                for oc in range(2):
                    ps[tcn, oc] = pspool.tile([128, 512], f32, tag="ps",
                                              name=f"ps_{tcn}_{oc}")

            def dense16_pass(ic, oc, start):  # ic0 phase
                for tcn in range(n_tc):
                    c0 = tcn * 512
                    for di in range(nd16):
                        o = DOFF16[di] + c0
                        nc.tensor.matmul(
                            ps[tcn, oc][:], kt_t[:, ic, oc, di, :],
                            xp[:, ic, o:o + 512],
                            start=(start and di == 0), stop=False,
                        )

            def tile_close(tcn, oc):
                c0 = tcn * 512
                ocs = slice(oc * 128, (oc + 1) * 128)
                last = (tcn == n_tc - 1 and oc == 1)
                # light taps: one K=256 fp8 DoubleRow matmul each
                for di in range(nd8):
                    o = DOFF8[di] + c0
                    nc.tensor.matmul(
                        ps[tcn, oc][:], kt8_t[:, :, oc, di, :],
                        x8_t[:, :, o:o + 512],
                        start=False, stop=False,
                        perf_mode=mybir.MatmulPerfMode.DoubleRow,
                    )
                for g in range(n_sg):
                    nc.tensor.matmul(
                        ps[tcn, oc][:], kp_t[g][:, ocs],
                        xg_t[g][:, c0:c0 + 512],
                        start=False, stop=(g == n_sg - 1),
                    )
                ot = opool.tile([128, 512], f16, tag="ot",
                                name=f"ot_{tcn}_{oc}")
                if not last:
                    if tcn % 2 == 0:
                        nc.vector.tensor_copy(ot[:], ps[tcn, oc][:])
                    else:
                        nc.scalar.activation(
                            ot[:], ps[tcn, oc][:],
                            mybir.ActivationFunctionType.Copy)
                    deng = (nc.gpsimd, nc.sync, nc.scalar)[(oc * n_tc + tcn) % 3]
                    deng.dma_start(y_d.ap()[oc][:, c0:c0 + 512], ot[:])
                else:
                    # split the final copy+store to trim the tail
                    nc.vector.tensor_copy(ot[:, 0:256], ps[tcn, oc][:, 0:256])
                    nc.scalar.activation(
                        ot[:, 256:512], ps[tcn, oc][:, 256:512],
                        mybir.ActivationFunctionType.Copy)
                    nc.gpsimd.dma_start(
                        y_d.ap()[oc][:, c0:c0 + 256], ot[:, 0:256])
                    nc.sync.dma_start(
                        y_d.ap()[oc][:, c0 + 256:c0 + 512], ot[:, 256:512])

            # Phase 1: heavy-tap ic0 matmuls while ic1/fp8/strip inputs
            # stream in. Phase 2: per tile, heavy-tap ic1 + fp8 taps +
            # strip + close, so stores spread across the back half.
            dense16_pass(0, 0, True)
            dense16_pass(0, 1, True)
            for oc in range(2):
                for tcn in range(n_tc):
                    c0 = tcn * 512
                    for di in range(nd16):
                        o = DOFF16[di] + c0
                        nc.tensor.matmul(
                            ps[tcn, oc][:], kt_t[:, 1, oc, di, :],
                            xp[:, 1, o:o + 512],
                            start=False, stop=False,
                        )
                    tile_close(tcn, oc)

    nc.compile()
    return nc


def kernel(x, weight, P, bias):
    global LAST_EXEC_NS, LAST_TRACE_PATH, DOFF16, DOFF8
    x = np.ascontiguousarray(np.asarray(x, dtype=np.float32))
    bias = np.asarray(bias, dtype=np.float32)

    dmin, ktaps = _host_fold_kernel(weight, P)
    T = ktaps.shape[0]
    dense16, dense8, strips = _classify_taps(ktaps)
    nd16, nd8 = len(dense16), len(dense8)
    assert nd16 >= 1, "degenerate kernel"

    # strip groups: (tap, row) pairs + one bias row, <= 128 rows per group
    rows = [(t, int(r)) for t, rr in strips for r in rr] + [(-1, -1)]
    groups = [rows[i:i + 128] for i in range(0, len(rows), 128)]
    # pad each group to a 32-aligned row count: the on-device zero-fill
    # of the remaining partitions must start at a 32-aligned partition
    groups = [g + [(-2, -1)] * (-len(g) % 32) for g in groups]
    strip_sizes = tuple(len(g) for g in groups)

    DOFF16 = list(dense16)  # tap column offsets used at emission time
    DOFF8 = list(dense8)

    key = (T, tuple(dense16), tuple(dense8),
           tuple(t for t, _ in rows[:-1]), strip_sizes)
    if key not in _BUILD_CACHE:
        _BUILD_CACHE[key] = _build(T, nd16, nd8, strip_sizes)
    nc = _BUILD_CACHE[key]

    # host-side input packing -------------------------------------------
    W = LEN + T - 1
    zl = max(0, PAD - dmin)
    xs = max(0, dmin - PAD)
    xn = min(LEN - xs, W - zl)
    xpad = np.zeros((N_CORES, 2, 128, W), dtype=np.float32)
    xpad[:, :, :, zl:zl + xn] = (
        x.reshape(N_CORES, 2, 128, LEN)[:, :, :, xs:xs + xn])

    xT = np.ascontiguousarray(xpad.transpose(0, 2, 1, 3))  # (c, 128, 2, W)
    x16 = xT.astype(F16)
    kt = np.ascontiguousarray(
        ktaps[dense16].reshape(nd16, 2, 128, 2, 128).transpose(2, 1, 3, 0, 4)
    ).astype(F16)
    # kt8[k, ic-tile, oc, tap, m] = ktaps[tap][ic_tile*128+k, oc*128+m]*16
    kt8 = _to_e4m3(
        np.ascontiguousarray(
            ktaps[dense8].reshape(nd8, 2, 128, 2, 128).transpose(2, 1, 3, 0, 4)
        ) * W_SCALE
    ).view(np.uint8)

    flat_x = xpad.reshape(N_CORES, 256, W)
    kps, xgs = [], []
    for g in groups:
        sp = len(g)
        kp = np.zeros((128, OC), dtype=np.float32)
        xg = np.zeros((N_CORES, sp, LEN), dtype=np.float32)
        for p, (t_sp, r) in enumerate(g):
            if t_sp == -2:  # alignment padding, stays zero
                continue
            if t_sp < 0:  # bias row
                kp[p] = bias
                xg[:, p] = 1.0
            else:
                kp[p] = ktaps[t_sp][r]
                xg[:, p] = flat_x[:, r, t_sp:t_sp + LEN]
        kps.append(kp.astype(F16))
        xgs.append(xg.astype(F16))

    in_maps = []
    for c in range(N_CORES):
        m = {"x": x16[c], "kt": kt, "kt8": kt8}
        for g in range(len(groups)):
            m[f"kp{g}"] = kps[g]
            m[f"xg{g}"] = xgs[g][c]
        in_maps.append(m)

    kwargs = {}
    bass_utils.upload_artifacts = lambda tmpdir: tmpdir
    if TRACE:
        kwargs["trace"] = True
    res = None
    for attempt in range(3):
        try:
            res = bass_utils.run_bass_kernel_spmd(
                nc, in_maps, core_ids=list(range(N_CORES)), **kwargs
            )
            break
        except Exception:
            # occasional transient NRT_EXEC_UNIT_UNRECOVERABLE on this
            # fabric; give the device a moment to recover, then retry
            if attempt == 2:
                raise
            import time
            time.sleep(3.0)
    if TRACE:
        LAST_EXEC_NS = res.exec_time_ns
        if res.instructions_and_trace is not None:
            LAST_TRACE_PATH = res.instructions_and_trace[1]

    out = np.empty((N, OC, LEN), dtype=np.float32)
    for c in range(N_CORES):
        out[c] = res.results[c]["out"].reshape(OC, LEN).astype(np.float32)
    return out
